# revision 22
# baseline (speedup 1.0000x reference)
"""Trainium2 Bass kernel for nn_AdaptiveSynapticDelayAttention (B=8,S=2048,E=768,H=1).

Math notes
----------
* ``mscores`` in the reference is constant along the softmax (key) axis and
  softmax is shift-invariant, so the whole membrane-potential branch
  (``membrane_potentials``/``decay_constant``/``q.mean``) cannot change the
  output.  That removes the only cross-batch coupling -> pure data-parallel
  over batch: one batch element per NeuronCore, no collectives.
* ``delayed[t,n] = spikes[t-d[n], n]`` (0 for ``t<d[n]``) with per-channel
  delay ``d[n] in [1,15]``.  Sorting channels by delay on the host (a column
  permutation of spikes + matching row permutation of Wq/Wk/Wv columns keeps
  the output bit-identical in exact math) turns the gather into <=~20
  contiguous shifted DMA loads on device.
* softmax without max-subtraction: logits are ~N(0, 0.3) here (weights are
  0.02-scale), exp() is far from overflow; softmax(x) == exp(x)/sum(exp(x)).

Per-core layout (one batch element, everything bf16 except where noted)
----------------------------------------------------------------------
  xT   (768, 2048)  delayed^T, assembled in SBUF from shifted DMAs
  Q^T,K^T (768,2048) = (W[:,perm].T).T @ xT   (weights stationary)
  V    (2048, 768)  = xT.T @ (Wv[:,perm].T)   (xT stationary)
  per q-tile (512 queries):
    s^T[k,q] = K^T.T @ Q^T   (f32 psum), Exp -> bf16 SBUF
    denom[1,q] via ones[128,1] stationary matmuls accumulated over k-chunks
    P^T[j,q] += V-chunk.T @ exp-chunk  (f32 psum, 2 groups of 3 j-chunks)
    bcast recip via K=1 matmul, normalize on DVE -> bf16
    out^T[i,q] = Wo.T.T @ P^T + bo    (f32) -> DRAM (host transposes back)
"""

import math
from contextlib import ExitStack

import numpy as np
import ml_dtypes

import concourse.bass as bass
import concourse.tile as tile
from concourse import bacc, mybir
from concourse.bass_utils import run_bass_kernel_spmd


def _install_ntff_hook():
    """The image's ``antenv`` lacks ``axon_hooks``, so the boot-time NTFF
    profile-hook registration degraded silently and trace=True would be
    skipped.  Recreate the module + hook here; degrade silently on any
    failure (tracing is optional, correctness never depends on it)."""
    try:
        import sys
        import types

        import antenv

        if hasattr(antenv, "axon_hooks"):
            return
        mod = types.ModuleType("antenv.axon_hooks")
        state = {"hook": None}
        mod.set_axon_ntff_profile_hook = lambda h: state.__setitem__("hook", h)
        mod.get_axon_ntff_profile_hook = lambda: state["hook"]
        sys.modules["antenv.axon_hooks"] = mod
        antenv.axon_hooks = mod
        from trn_agent_boot.trn_boot import _ntff_profile_via_ctypes

        mod.set_axon_ntff_profile_hook(
            _ntff_profile_via_ctypes("/opt/axon/libaxon_pjrt.so"))
    except Exception:
        pass


_install_ntff_hook()

BF16 = mybir.dt.bfloat16
F32 = mybir.dt.float32

B, S, E = 8, 2048, 768
P = 128
EC = E // P          # 6 embed chunks
KC = S // P          # 16 key chunks
QT = 512             # q-tile width
NQT = S // QT        # 4 q tiles
TT = 512             # t-tile width for Q/K production
NTT = S // TT
VH = 384             # V produced in two 384-wide halves (one psum bank each)
INV_SQRT_HD = 1.0 / math.sqrt(float(E))

# set by test.py to capture a profiled run
TRACE = False
LAST_RESULT = None

_BUILD_CACHE: dict = {}


def _build(groups):
    """groups: tuple of (row0, row1, delay) covering rows [0, E) of xT."""
    nc = bacc.Bacc("TRN2", target_bir_lowering=False, debug=False, num_devices=8)

    xT_ext = nc.dram_tensor("xT", [E, S], BF16, kind="ExternalInput")
    wq_ext = nc.dram_tensor("wqT", [E, E], BF16, kind="ExternalInput")
    wk_ext = nc.dram_tensor("wkT", [E, E], BF16, kind="ExternalInput")
    wv_ext = nc.dram_tensor("wvT", [E, E], BF16, kind="ExternalInput")
    wo_ext = nc.dram_tensor("woT", [E, E], BF16, kind="ExternalInput")
    bo_ext = nc.dram_tensor("bo", [E, 1], F32, kind="ExternalInput")
    out_ext = nc.dram_tensor("out", [E, S], F32, kind="ExternalOutput")

    with tile.TileContext(nc) as tc, ExitStack() as ctx:
        pers = ctx.enter_context(tc.tile_pool(name="pers", bufs=1))
        expp = ctx.enter_context(tc.tile_pool(name="expp", bufs=20))
        ptp = ctx.enter_context(tc.tile_pool(name="ptp", bufs=8))
        otp = ctx.enter_context(tc.tile_pool(name="otp", bufs=3))
        smallp = ctx.enter_context(tc.tile_pool(name="smallp", bufs=2))
        psp = ctx.enter_context(tc.tile_pool(name="psp", bufs=1, space="PSUM"))

        # ---- persistent SBUF tensors ----
        xt = [pers.tile([P, S], BF16, name=f"xt{c}", tag=f"xt{c}") for c in range(EC)]
        wq = [pers.tile([P, E], BF16, name=f"wq{c}", tag=f"wq{c}") for c in range(EC)]
        wk = [pers.tile([P, E], BF16, name=f"wk{c}", tag=f"wk{c}") for c in range(EC)]
        wv = [pers.tile([P, E], BF16, name=f"wv{c}", tag=f"wv{c}") for c in range(EC)]
        wo = [pers.tile([P, E], BF16, name=f"wo{c}", tag=f"wo{c}") for c in range(EC)]
        qt_sb = [pers.tile([P, S], BF16, name=f"qt{c}", tag=f"qt{c}") for c in range(EC)]
        kt_sb = [pers.tile([P, S], BF16, name=f"kt{c}", tag=f"kt{c}") for c in range(EC)]
        v_sb = [pers.tile([P, E], BF16, name=f"v{t}", tag=f"v{t}") for t in range(KC)]
        bo_sb = pers.tile([P, EC], F32, name="bo_sb", tag="bo_sb")
        # [128,128] of ones: the denominator matmuls then produce the column
        # sums already broadcast across all 128 partitions (M does not affect
        # matmul stream time), so normalization needs only one [128,512]
        # reciprocal afterwards.
        ones_bf = pers.tile([P, P], BF16, name="ones_bf", tag="ones_bf")

        nc.vector.memset(ones_bf[:, :], 1.0)
        warm_rhs = pers.tile([P, QT], BF16, name="warm_rhs", tag="warm_rhs")
        nc.vector.memset(warm_rhs[:, :], 0.0)
        # ~120 zero matmuls keep the PE busy while the inputs stream in, so
        # the HAM clock-gate is already released (K=8/8) when the first real
        # matmul issues; they cycle through the same psum ring ahead of the
        # real groups and finish before the xt tail arrives.
        for _ in range(120):
            pw = psp.tile([P, QT], F32, name="mmps", tag="mmps", bufs=2)
            nc.tensor.matmul(pw[:, :], lhsT=ones_bf[:, :], rhs=warm_rhs[:, :],
                             start=True, stop=True)
        # wq on the gpsimd SWDGE ring first (first matmuls need it with xt);
        # the rest of the weights queue behind the gpsimd-shard of xt loads.
        for c in range(EC):
            nc.gpsimd.dma_start(out=wq[c][:, :], in_=wq_ext[c * P:(c + 1) * P, :])

        # ---- assemble delayed^T: per delay-group shifted loads ----
        # Compute engines need 32-aligned partition bases, so zero the full
        # [0:dmax) margin per chunk first; the shifted DMAs (arbitrary
        # partition ranges are fine for DMA) overwrite [d:) per group and
        # program order resolves the overlap in the DMA's favor.  Loads
        # alternate over both HWDGE rings; ACT carries no compute during the
        # load window (Q/K/V psum copies all run on DVE) so both rings drain
        # back-to-back.
        dmax = [0] * EC
        chunk_groups: list[list[tuple]] = [[] for _ in range(EC)]
        for (r0, r1, dv) in groups:
            while r0 < r1:
                c = r0 // P
                seg_end = min(r1, (c + 1) * P)
                d = int(min(dv, S))
                dmax[c] = max(dmax[c], d)
                chunk_groups[c].append((r0 - c * P, seg_end - c * P, d))
                r0 = seg_end
        for c in range(EC):
            if dmax[c] > 0:
                nc.vector.memset(xt[c][:, 0:dmax[c]], 0.0)
        # balance the shifted loads by row count (descriptor count is the
        # binding resource); the SWDGE ring gets half weight.
        engs = (nc.sync, nc.scalar, nc.gpsimd)
        speed = (1.0, 1.0, 0.5)
        load = [0.0, 0.0, 0.0]
        segs = [(c, p0, p1, d) for c in range(EC)
                for (p0, p1, d) in chunk_groups[c] if d < S]
        for (c, p0, p1, d) in segs:
            r = min(range(3), key=lambda i: (load[i] + (p1 - p0)) / speed[i])
            load[r] += p1 - p0
            engs[r].dma_start(out=xt[c][p0:p1, d:S],
                              in_=xT_ext[c * P + p0:c * P + p1, 0:S - d])
        for c in range(EC):
            nc.gpsimd.dma_start(out=wk[c][:, :], in_=wk_ext[c * P:(c + 1) * P, :])
        for c in range(EC):
            nc.gpsimd.dma_start(out=wv[c][:, :], in_=wv_ext[c * P:(c + 1) * P, :])
            nc.gpsimd.dma_start(out=wo[c][:, :], in_=wo_ext[c * P:(c + 1) * P, :])
            nc.gpsimd.dma_start(out=bo_sb[:, c:c + 1], in_=bo_ext[c * P:(c + 1) * P, :])

        # ---- Q^T, K^T: out[i,t] chunks; weights stationary ----
        for (w, dst, scale) in ((wq, qt_sb, INV_SQRT_HD), (wk, kt_sb, 1.0)):
            for ic in range(EC):
                for tt in range(NTT):
                    ps = psp.tile([P, TT], F32, name="mmps", tag="mmps", bufs=2)
                    for jc in range(EC):
                        nc.tensor.matmul(
                            ps[:, :],
                            lhsT=w[jc][:, ic * P:(ic + 1) * P],
                            rhs=xt[jc][:, tt * TT:(tt + 1) * TT],
                            start=(jc == 0),
                            stop=(jc == EC - 1),
                        )
                    if scale != 1.0:
                        nc.vector.tensor_scalar_mul(
                            dst[ic][:, tt * TT:(tt + 1) * TT], ps[:, :], scale)
                    else:
                        nc.vector.tensor_copy(
                            out=dst[ic][:, tt * TT:(tt + 1) * TT], in_=ps[:, :],
                        )

        # ---- V (normal orientation): xT stationary, wv moving ----
        for t in range(KC):
            for h in range(2):
                psv = psp.tile([P, VH], F32, name="mmps", tag="mmps", bufs=2)
                for jc in range(EC):
                    nc.tensor.matmul(
                        psv[:, :],
                        lhsT=xt[jc][:, t * P:(t + 1) * P],
                        rhs=wv[jc][:, h * VH:(h + 1) * VH],
                        start=(jc == 0),
                        stop=(jc == EC - 1),
                    )
                nc.vector.tensor_copy(out=v_sb[t][:, h * VH:(h + 1) * VH], in_=psv[:, :])

        # ---- attention, one 512-wide q-tile at a time ----
        for q in range(NQT):
            q0 = q * QT
            exp_tiles = []
            psS = psp.tile([P, QT], F32, name="scl", tag="scl", bufs=2)
            for kc in range(KC):
                ps = psp.tile([P, QT], F32, name="mmps", tag="mmps", bufs=2)
                for ic in range(EC):
                    nc.tensor.matmul(
                        ps[:, :],
                        lhsT=kt_sb[ic][:, kc * P:(kc + 1) * P],
                        rhs=qt_sb[ic][:, q0:q0 + QT],
                        start=(ic == 0),
                        stop=(ic == EC - 1),
                    )
                e = expp.tile([P, QT], BF16, name="exp", tag="exp", bufs=20)
                nc.scalar.activation(
                    out=e[:, :], in_=ps[:, :], func=mybir.ActivationFunctionType.Exp,
                )
                exp_tiles.append(e)
                nc.tensor.matmul(
                    psS[:, :], lhsT=ones_bf[:, :], rhs=e[:, :],
                    start=(kc == 0), stop=(kc == KC - 1),
                )

            # psS holds the denominators broadcast over all partitions; one
            # reciprocal lands the normalizer in SBUF (DVE can read only one
            # PSUM operand in the muls below).
            bc_sb = smallp.tile([P, QT], F32, name="bc_sb", tag="bc_sb", bufs=2)
            nc.vector.reciprocal(bc_sb[:, :], psS[:, :])

            pt_tiles = []
            for jc in range(EC):
                pv = psp.tile([P, QT], F32, name="pav", tag="pav", bufs=2)
                for kc in range(KC):
                    nc.tensor.matmul(
                        pv[:, :],
                        lhsT=v_sb[kc][:, jc * P:(jc + 1) * P],
                        rhs=exp_tiles[kc][:, :],
                        start=(kc == 0),
                        stop=(kc == KC - 1),
                    )
                pt = ptp.tile([P, QT], BF16, name="pt", tag="pt", bufs=8)
                nc.vector.tensor_mul(pt[:, :], pv[:, :], bc_sb[:, :])
                pt_tiles.append(pt)

            for ic in range(EC):
                po = psp.tile([P, QT], F32, name="po", tag="po", bufs=2)
                for jc in range(EC):
                    nc.tensor.matmul(
                        po[:, :],
                        lhsT=wo[jc][:, ic * P:(ic + 1) * P],
                        rhs=pt_tiles[jc][:, :],
                        start=(jc == 0),
                        stop=(jc == EC - 1),
                    )
                ot = otp.tile([P, QT], F32, name="ot", tag="ot", bufs=3)
                if q == NQT - 1:
                    nc.scalar.activation(
                        out=ot[:, :], in_=po[:, :],
                        func=mybir.ActivationFunctionType.Identity,
                        bias=bo_sb[:, ic:ic + 1])
                else:
                    nc.vector.tensor_scalar_add(ot[:, :], po[:, :],
                                                bo_sb[:, ic:ic + 1])
                oeng = nc.sync if ic % 2 == 0 else nc.scalar
                oeng.dma_start(out=out_ext[ic * P:(ic + 1) * P, q0:q0 + QT],
                               in_=ot[:, :])

    nc.compile()
    return nc


def _delay_groups(d_sorted):
    groups = []
    r0 = 0
    for r in range(1, len(d_sorted) + 1):
        if r == len(d_sorted) or d_sorted[r] != d_sorted[r0]:
            groups.append((r0, r, int(d_sorted[r0])))
            r0 = r
    return tuple(groups)


def kernel(**inputs) -> np.ndarray:
    global LAST_RESULT
    spikes = np.asarray(inputs["spikes"], dtype=np.float32)
    dw = np.asarray(inputs["delay_weights"]).reshape(-1).astype(np.int64)
    Wq = np.asarray(inputs["Wq"], dtype=np.float32)
    Wk = np.asarray(inputs["Wk"], dtype=np.float32)
    Wv = np.asarray(inputs["Wv"], dtype=np.float32)
    Wo = np.asarray(inputs["Wo"], dtype=np.float32)
    bo = np.asarray(inputs["bo"], dtype=np.float32)

    order = np.argsort(dw, kind="stable")
    groups = _delay_groups(dw[order])

    key = groups
    if key not in _BUILD_CACHE:
        _BUILD_CACHE[key] = _build(groups)
    nc = _BUILD_CACHE[key]

    bf = ml_dtypes.bfloat16
    wqT = np.ascontiguousarray(Wq[:, order].T).astype(bf)
    wkT = np.ascontiguousarray(Wk[:, order].T).astype(bf)
    wvT = np.ascontiguousarray(Wv[:, order].T).astype(bf)
    woT = np.ascontiguousarray(Wo.T).astype(bf)
    bo2 = np.ascontiguousarray(bo.reshape(E, 1))

    in_maps = []
    for b in range(B):
        xT = np.ascontiguousarray(spikes[b].T[order]).astype(bf)
        in_maps.append({"xT": xT, "wqT": wqT, "wkT": wkT, "wvT": wvT,
                        "woT": woT, "bo": bo2})

    LAST_RESULT = run_bass_kernel_spmd(
        nc, in_maps, core_ids=list(range(B)), trace=TRACE,
    )
    out = np.stack([LAST_RESULT.results[b]["out"].T for b in range(B)])
    return np.ascontiguousarray(out.astype(np.float32))


# revision 23
# speedup vs baseline: 1.0003x; 1.0003x over previous
"""Trainium2 Bass kernel for nn_AdaptiveSynapticDelayAttention (B=8,S=2048,E=768,H=1).

Math notes
----------
* ``mscores`` in the reference is constant along the softmax (key) axis and
  softmax is shift-invariant, so the whole membrane-potential branch
  (``membrane_potentials``/``decay_constant``/``q.mean``) cannot change the
  output.  That removes the only cross-batch coupling -> pure data-parallel
  over batch: one batch element per NeuronCore, no collectives.
* ``delayed[t,n] = spikes[t-d[n], n]`` (0 for ``t<d[n]``) with per-channel
  delay ``d[n] in [1,15]``.  Sorting channels by delay on the host (a column
  permutation of spikes + matching row permutation of Wq/Wk/Wv columns keeps
  the output bit-identical in exact math) turns the gather into <=~20
  contiguous shifted DMA loads on device.
* softmax without max-subtraction: logits are ~N(0, 0.3) here (weights are
  0.02-scale), exp() is far from overflow; softmax(x) == exp(x)/sum(exp(x)).

Per-core layout (one batch element, everything bf16 except where noted)
----------------------------------------------------------------------
  xT   (768, 2048)  delayed^T, assembled in SBUF from shifted DMAs
  Q^T,K^T (768,2048) = (W[:,perm].T).T @ xT   (weights stationary)
  V    (2048, 768)  = xT.T @ (Wv[:,perm].T)   (xT stationary)
  per q-tile (512 queries):
    s^T[k,q] = K^T.T @ Q^T   (f32 psum), Exp -> bf16 SBUF
    denom[1,q] via ones[128,1] stationary matmuls accumulated over k-chunks
    P^T[j,q] += V-chunk.T @ exp-chunk  (f32 psum, 2 groups of 3 j-chunks)
    bcast recip via K=1 matmul, normalize on DVE -> bf16
    out^T[i,q] = Wo.T.T @ P^T + bo    (f32) -> DRAM (host transposes back)
"""

import math
from contextlib import ExitStack

import numpy as np
import ml_dtypes

import concourse.bass as bass
import concourse.tile as tile
from concourse import bacc, mybir
from concourse.bass_utils import run_bass_kernel_spmd


def _install_ntff_hook():
    """The image's ``antenv`` lacks ``axon_hooks``, so the boot-time NTFF
    profile-hook registration degraded silently and trace=True would be
    skipped.  Recreate the module + hook here; degrade silently on any
    failure (tracing is optional, correctness never depends on it)."""
    try:
        import sys
        import types

        import antenv

        if hasattr(antenv, "axon_hooks"):
            return
        mod = types.ModuleType("antenv.axon_hooks")
        state = {"hook": None}
        mod.set_axon_ntff_profile_hook = lambda h: state.__setitem__("hook", h)
        mod.get_axon_ntff_profile_hook = lambda: state["hook"]
        sys.modules["antenv.axon_hooks"] = mod
        antenv.axon_hooks = mod
        from trn_agent_boot.trn_boot import _ntff_profile_via_ctypes

        mod.set_axon_ntff_profile_hook(
            _ntff_profile_via_ctypes("/opt/axon/libaxon_pjrt.so"))
    except Exception:
        pass


_install_ntff_hook()

BF16 = mybir.dt.bfloat16
F32 = mybir.dt.float32

B, S, E = 8, 2048, 768
P = 128
EC = E // P          # 6 embed chunks
KC = S // P          # 16 key chunks
QT = 512             # q-tile width
NQT = S // QT        # 4 q tiles
TT = 512             # t-tile width for Q/K production
NTT = S // TT
VH = 384             # V produced in two 384-wide halves (one psum bank each)
INV_SQRT_HD = 1.0 / math.sqrt(float(E))

# set by test.py to capture a profiled run
TRACE = False
LAST_RESULT = None

_BUILD_CACHE: dict = {}


def _build(groups):
    """groups: tuple of (row0, row1, delay) covering rows [0, E) of xT."""
    nc = bacc.Bacc("TRN2", target_bir_lowering=False, debug=False, num_devices=8)

    xT_ext = nc.dram_tensor("xT", [E, S], BF16, kind="ExternalInput")
    wq_ext = nc.dram_tensor("wqT", [E, E], BF16, kind="ExternalInput")
    wk_ext = nc.dram_tensor("wkT", [E, E], BF16, kind="ExternalInput")
    wv_ext = nc.dram_tensor("wvT", [E, E], BF16, kind="ExternalInput")
    wo_ext = nc.dram_tensor("woT", [E, E], BF16, kind="ExternalInput")
    bo_ext = nc.dram_tensor("bo", [E, 1], F32, kind="ExternalInput")
    out_ext = nc.dram_tensor("out", [E, S], F32, kind="ExternalOutput")

    with tile.TileContext(nc) as tc, ExitStack() as ctx:
        pers = ctx.enter_context(tc.tile_pool(name="pers", bufs=1))
        expp = ctx.enter_context(tc.tile_pool(name="expp", bufs=20))
        ptp = ctx.enter_context(tc.tile_pool(name="ptp", bufs=8))
        otp = ctx.enter_context(tc.tile_pool(name="otp", bufs=3))
        smallp = ctx.enter_context(tc.tile_pool(name="smallp", bufs=2))
        psp = ctx.enter_context(tc.tile_pool(name="psp", bufs=1, space="PSUM"))

        # ---- persistent SBUF tensors ----
        xt = [pers.tile([P, S], BF16, name=f"xt{c}", tag=f"xt{c}") for c in range(EC)]
        wq = [pers.tile([P, E], BF16, name=f"wq{c}", tag=f"wq{c}") for c in range(EC)]
        wk = [pers.tile([P, E], BF16, name=f"wk{c}", tag=f"wk{c}") for c in range(EC)]
        wv = [pers.tile([P, E], BF16, name=f"wv{c}", tag=f"wv{c}") for c in range(EC)]
        wo = [pers.tile([P, E], BF16, name=f"wo{c}", tag=f"wo{c}") for c in range(EC)]
        qt_sb = [pers.tile([P, S], BF16, name=f"qt{c}", tag=f"qt{c}") for c in range(EC)]
        kt_sb = [pers.tile([P, S], BF16, name=f"kt{c}", tag=f"kt{c}") for c in range(EC)]
        v_sb = [pers.tile([P, E], BF16, name=f"v{t}", tag=f"v{t}") for t in range(KC)]
        bo_sb = pers.tile([P, EC], F32, name="bo_sb", tag="bo_sb")
        # [128,128] of ones: the denominator matmuls then produce the column
        # sums already broadcast across all 128 partitions (M does not affect
        # matmul stream time), so normalization needs only one [128,512]
        # reciprocal afterwards.
        ones_bf = pers.tile([P, P], BF16, name="ones_bf", tag="ones_bf")

        nc.vector.memset(ones_bf[:, :], 1.0)
        # wq on the gpsimd SWDGE ring first (first matmuls need it with xt);
        # the rest of the weights queue behind the gpsimd-shard of xt loads.
        for c in range(EC):
            nc.gpsimd.dma_start(out=wq[c][:, :], in_=wq_ext[c * P:(c + 1) * P, :])

        # ---- assemble delayed^T: per delay-group shifted loads ----
        # Compute engines need 32-aligned partition bases, so zero the full
        # [0:dmax) margin per chunk first; the shifted DMAs (arbitrary
        # partition ranges are fine for DMA) overwrite [d:) per group and
        # program order resolves the overlap in the DMA's favor.  Loads
        # alternate over both HWDGE rings; ACT carries no compute during the
        # load window (Q/K/V psum copies all run on DVE) so both rings drain
        # back-to-back.
        dmax = [0] * EC
        chunk_groups: list[list[tuple]] = [[] for _ in range(EC)]
        for (r0, r1, dv) in groups:
            while r0 < r1:
                c = r0 // P
                seg_end = min(r1, (c + 1) * P)
                d = int(min(dv, S))
                dmax[c] = max(dmax[c], d)
                chunk_groups[c].append((r0 - c * P, seg_end - c * P, d))
                r0 = seg_end
        for c in range(EC):
            if dmax[c] > 0:
                nc.vector.memset(xt[c][:, 0:dmax[c]], 0.0)
        # balance the shifted loads by row count (descriptor count is the
        # binding resource); the SWDGE ring gets half weight.
        engs = (nc.sync, nc.scalar, nc.gpsimd)
        speed = (1.0, 1.0, 0.5)
        load = [0.0, 0.0, 0.0]
        segs = [(c, p0, p1, d) for c in range(EC)
                for (p0, p1, d) in chunk_groups[c] if d < S]
        for (c, p0, p1, d) in segs:
            r = min(range(3), key=lambda i: (load[i] + (p1 - p0)) / speed[i])
            load[r] += p1 - p0
            engs[r].dma_start(out=xt[c][p0:p1, d:S],
                              in_=xT_ext[c * P + p0:c * P + p1, 0:S - d])
        for c in range(EC):
            nc.gpsimd.dma_start(out=wk[c][:, :], in_=wk_ext[c * P:(c + 1) * P, :])
        for c in range(EC):
            nc.gpsimd.dma_start(out=wv[c][:, :], in_=wv_ext[c * P:(c + 1) * P, :])
            nc.gpsimd.dma_start(out=wo[c][:, :], in_=wo_ext[c * P:(c + 1) * P, :])
            nc.gpsimd.dma_start(out=bo_sb[:, c:c + 1], in_=bo_ext[c * P:(c + 1) * P, :])

        # ---- Q^T, K^T: out[i,t] chunks; weights stationary ----
        for (w, dst, scale) in ((wq, qt_sb, INV_SQRT_HD), (wk, kt_sb, 1.0)):
            for ic in range(EC):
                for tt in range(NTT):
                    ps = psp.tile([P, TT], F32, name="mmps", tag="mmps", bufs=2)
                    for jc in range(EC):
                        nc.tensor.matmul(
                            ps[:, :],
                            lhsT=w[jc][:, ic * P:(ic + 1) * P],
                            rhs=xt[jc][:, tt * TT:(tt + 1) * TT],
                            start=(jc == 0),
                            stop=(jc == EC - 1),
                        )
                    if scale != 1.0:
                        nc.vector.tensor_scalar_mul(
                            dst[ic][:, tt * TT:(tt + 1) * TT], ps[:, :], scale)
                    else:
                        nc.vector.tensor_copy(
                            out=dst[ic][:, tt * TT:(tt + 1) * TT], in_=ps[:, :],
                        )

        # ---- V (normal orientation): xT stationary, wv moving ----
        for t in range(KC):
            for h in range(2):
                psv = psp.tile([P, VH], F32, name="mmps", tag="mmps", bufs=2)
                for jc in range(EC):
                    nc.tensor.matmul(
                        psv[:, :],
                        lhsT=xt[jc][:, t * P:(t + 1) * P],
                        rhs=wv[jc][:, h * VH:(h + 1) * VH],
                        start=(jc == 0),
                        stop=(jc == EC - 1),
                    )
                nc.vector.tensor_copy(out=v_sb[t][:, h * VH:(h + 1) * VH], in_=psv[:, :])

        # ---- attention, one 512-wide q-tile at a time ----
        for q in range(NQT):
            q0 = q * QT
            exp_tiles = []
            psS = psp.tile([P, QT], F32, name="scl", tag="scl", bufs=2)
            for kc in range(KC):
                ps = psp.tile([P, QT], F32, name="mmps", tag="mmps", bufs=2)
                for ic in range(EC):
                    nc.tensor.matmul(
                        ps[:, :],
                        lhsT=kt_sb[ic][:, kc * P:(kc + 1) * P],
                        rhs=qt_sb[ic][:, q0:q0 + QT],
                        start=(ic == 0),
                        stop=(ic == EC - 1),
                    )
                e = expp.tile([P, QT], BF16, name="exp", tag="exp", bufs=20)
                nc.scalar.activation(
                    out=e[:, :], in_=ps[:, :], func=mybir.ActivationFunctionType.Exp,
                )
                exp_tiles.append(e)
                nc.tensor.matmul(
                    psS[:, :], lhsT=ones_bf[:, :], rhs=e[:, :],
                    start=(kc == 0), stop=(kc == KC - 1),
                )

            # psS holds the denominators broadcast over all partitions; one
            # reciprocal lands the normalizer in SBUF (DVE can read only one
            # PSUM operand in the muls below).
            bc_sb = smallp.tile([P, QT], F32, name="bc_sb", tag="bc_sb", bufs=2)
            nc.vector.reciprocal(bc_sb[:, :], psS[:, :])

            pt_tiles = []
            for jc in range(EC):
                pv = psp.tile([P, QT], F32, name="pav", tag="pav", bufs=2)
                for kc in range(KC):
                    nc.tensor.matmul(
                        pv[:, :],
                        lhsT=v_sb[kc][:, jc * P:(jc + 1) * P],
                        rhs=exp_tiles[kc][:, :],
                        start=(kc == 0),
                        stop=(kc == KC - 1),
                    )
                pt = ptp.tile([P, QT], BF16, name="pt", tag="pt", bufs=8)
                nc.vector.tensor_mul(pt[:, :], pv[:, :], bc_sb[:, :])
                pt_tiles.append(pt)

            for ic in range(EC):
                po = psp.tile([P, QT], F32, name="po", tag="po", bufs=2)
                for jc in range(EC):
                    nc.tensor.matmul(
                        po[:, :],
                        lhsT=wo[jc][:, ic * P:(ic + 1) * P],
                        rhs=pt_tiles[jc][:, :],
                        start=(jc == 0),
                        stop=(jc == EC - 1),
                    )
                ot = otp.tile([P, QT], F32, name="ot", tag="ot", bufs=3)
                if q == NQT - 1:
                    nc.scalar.activation(
                        out=ot[:, :], in_=po[:, :],
                        func=mybir.ActivationFunctionType.Identity,
                        bias=bo_sb[:, ic:ic + 1])
                else:
                    nc.vector.tensor_scalar_add(ot[:, :], po[:, :],
                                                bo_sb[:, ic:ic + 1])
                oeng = nc.sync if ic % 2 == 0 else nc.scalar
                oeng.dma_start(out=out_ext[ic * P:(ic + 1) * P, q0:q0 + QT],
                               in_=ot[:, :])

    nc.compile()
    return nc


def _delay_groups(d_sorted):
    groups = []
    r0 = 0
    for r in range(1, len(d_sorted) + 1):
        if r == len(d_sorted) or d_sorted[r] != d_sorted[r0]:
            groups.append((r0, r, int(d_sorted[r0])))
            r0 = r
    return tuple(groups)


def kernel(**inputs) -> np.ndarray:
    global LAST_RESULT
    spikes = np.asarray(inputs["spikes"], dtype=np.float32)
    dw = np.asarray(inputs["delay_weights"]).reshape(-1).astype(np.int64)
    Wq = np.asarray(inputs["Wq"], dtype=np.float32)
    Wk = np.asarray(inputs["Wk"], dtype=np.float32)
    Wv = np.asarray(inputs["Wv"], dtype=np.float32)
    Wo = np.asarray(inputs["Wo"], dtype=np.float32)
    bo = np.asarray(inputs["bo"], dtype=np.float32)

    order = np.argsort(dw, kind="stable")
    groups = _delay_groups(dw[order])

    key = groups
    if key not in _BUILD_CACHE:
        _BUILD_CACHE[key] = _build(groups)
    nc = _BUILD_CACHE[key]

    bf = ml_dtypes.bfloat16
    wqT = np.ascontiguousarray(Wq[:, order].T).astype(bf)
    wkT = np.ascontiguousarray(Wk[:, order].T).astype(bf)
    wvT = np.ascontiguousarray(Wv[:, order].T).astype(bf)
    woT = np.ascontiguousarray(Wo.T).astype(bf)
    bo2 = np.ascontiguousarray(bo.reshape(E, 1))

    in_maps = []
    for b in range(B):
        xT = np.ascontiguousarray(spikes[b].T[order]).astype(bf)
        in_maps.append({"xT": xT, "wqT": wqT, "wkT": wkT, "wvT": wvT,
                        "woT": woT, "bo": bo2})

    LAST_RESULT = run_bass_kernel_spmd(
        nc, in_maps, core_ids=list(range(B)), trace=TRACE,
    )
    out = np.stack([LAST_RESULT.results[b]["out"].T for b in range(B)])
    return np.ascontiguousarray(out.astype(np.float32))


# revision 24
# speedup vs baseline: 1.0570x; 1.0567x over previous
"""Trainium2 Bass kernel for nn_AdaptiveSynapticDelayAttention (B=8,S=2048,E=768,H=1).

Math notes
----------
* ``mscores`` in the reference is constant along the softmax (key) axis and
  softmax is shift-invariant, so the whole membrane-potential branch
  (``membrane_potentials``/``decay_constant``/``q.mean``) cannot change the
  output.  That removes the only cross-batch coupling -> pure data-parallel
  over batch: one batch element per NeuronCore, no collectives.
* ``delayed[t,n] = spikes[t-d[n], n]`` (0 for ``t<d[n]``) with per-channel
  delay ``d[n] in [1,15]``.  Sorting channels by delay on the host (a column
  permutation of spikes + matching row permutation of Wq/Wk/Wv columns keeps
  the output bit-identical in exact math) turns the gather into <=~20
  contiguous shifted DMA loads on device.
* softmax without max-subtraction: logits are ~N(0, 0.3) here (weights are
  0.02-scale), exp() is far from overflow; softmax(x) == exp(x)/sum(exp(x)).

Per-core layout (one batch element, everything bf16 except where noted)
----------------------------------------------------------------------
  xT   (768, 2048)  delayed^T, assembled in SBUF from shifted DMAs
  Q^T,K^T (768,2048) = (W[:,perm].T).T @ xT   (weights stationary)
  V    (2048, 768)  = xT.T @ (Wv[:,perm].T)   (xT stationary)
  per q-tile (512 queries):
    s^T[k,q] = K^T.T @ Q^T   (f32 psum), Exp -> bf16 SBUF
    denom[1,q] via ones[128,1] stationary matmuls accumulated over k-chunks
    P^T[j,q] += V-chunk.T @ exp-chunk  (f32 psum, 2 groups of 3 j-chunks)
    bcast recip via K=1 matmul, normalize on DVE -> bf16
    out^T[i,q] = Wo.T.T @ P^T + bo    (f32) -> DRAM (host transposes back)
"""

import math
from contextlib import ExitStack

import numpy as np
import ml_dtypes

import concourse.bass as bass
import concourse.tile as tile
from concourse import bacc, mybir
from concourse.bass_utils import run_bass_kernel_spmd


def _install_ntff_hook():
    """The image's ``antenv`` lacks ``axon_hooks``, so the boot-time NTFF
    profile-hook registration degraded silently and trace=True would be
    skipped.  Recreate the module + hook here; degrade silently on any
    failure (tracing is optional, correctness never depends on it)."""
    try:
        import sys
        import types

        import antenv

        if hasattr(antenv, "axon_hooks"):
            return
        mod = types.ModuleType("antenv.axon_hooks")
        state = {"hook": None}
        mod.set_axon_ntff_profile_hook = lambda h: state.__setitem__("hook", h)
        mod.get_axon_ntff_profile_hook = lambda: state["hook"]
        sys.modules["antenv.axon_hooks"] = mod
        antenv.axon_hooks = mod
        from trn_agent_boot.trn_boot import _ntff_profile_via_ctypes

        mod.set_axon_ntff_profile_hook(
            _ntff_profile_via_ctypes("/opt/axon/libaxon_pjrt.so"))
    except Exception:
        pass


_install_ntff_hook()

BF16 = mybir.dt.bfloat16
F32 = mybir.dt.float32

B, S, E = 8, 2048, 768
P = 128
EC = E // P          # 6 embed chunks
KC = S // P          # 16 key chunks
QT = 512             # q-tile width
NQT = S // QT        # 4 q tiles
TT = 512             # t-tile width for Q/K production
NTT = S // TT
VH = 384             # V produced in two 384-wide halves (one psum bank each)
INV_SQRT_HD = 1.0 / math.sqrt(float(E))

# set by test.py to capture a profiled run
TRACE = False
LAST_RESULT = None

_BUILD_CACHE: dict = {}


def _build(groups):
    """groups: tuple of (row0, row1, delay) covering rows [0, E) of xT."""
    nc = bacc.Bacc("TRN2", target_bir_lowering=False, debug=False, num_devices=8)

    xT_ext = nc.dram_tensor("xT", [E, S], BF16, kind="ExternalInput")
    wq_ext = nc.dram_tensor("wqT", [E, E], BF16, kind="ExternalInput")
    wk_ext = nc.dram_tensor("wkT", [E, E], BF16, kind="ExternalInput")
    wv_ext = nc.dram_tensor("wvT", [E, E], BF16, kind="ExternalInput")
    wo_ext = nc.dram_tensor("woT", [E, E], BF16, kind="ExternalInput")
    bo_ext = nc.dram_tensor("bo", [E, 1], F32, kind="ExternalInput")
    out_ext = nc.dram_tensor("out", [E, S], F32, kind="ExternalOutput")

    with tile.TileContext(nc) as tc, ExitStack() as ctx:
        pers = ctx.enter_context(tc.tile_pool(name="pers", bufs=1))
        expp = ctx.enter_context(tc.tile_pool(name="expp", bufs=20))
        ptp = ctx.enter_context(tc.tile_pool(name="ptp", bufs=8))
        otp = ctx.enter_context(tc.tile_pool(name="otp", bufs=3))
        smallp = ctx.enter_context(tc.tile_pool(name="smallp", bufs=2))
        psp = ctx.enter_context(tc.tile_pool(name="psp", bufs=1, space="PSUM"))

        # ---- persistent SBUF tensors ----
        xt = [pers.tile([P, S], BF16, name=f"xt{c}", tag=f"xt{c}") for c in range(EC)]
        wq = [pers.tile([P, E], BF16, name=f"wq{c}", tag=f"wq{c}") for c in range(EC)]
        wk = [pers.tile([P, E], BF16, name=f"wk{c}", tag=f"wk{c}") for c in range(EC)]
        wv = [pers.tile([P, E], BF16, name=f"wv{c}", tag=f"wv{c}") for c in range(EC)]
        wo = [pers.tile([P, E], BF16, name=f"wo{c}", tag=f"wo{c}") for c in range(EC)]
        qt_sb = [pers.tile([P, S], BF16, name=f"qt{c}", tag=f"qt{c}") for c in range(EC)]
        kt_sb = [pers.tile([P, S], BF16, name=f"kt{c}", tag=f"kt{c}") for c in range(EC)]
        v_sb = [pers.tile([P, E], BF16, name=f"v{t}", tag=f"v{t}") for t in range(KC)]
        bo_sb = pers.tile([P, EC], F32, name="bo_sb", tag="bo_sb")
        # [128,128] of ones: the denominator matmuls then produce the column
        # sums already broadcast across all 128 partitions (M does not affect
        # matmul stream time), so normalization needs only one [128,512]
        # reciprocal afterwards.
        ones_bf = pers.tile([P, P], BF16, name="ones_bf", tag="ones_bf")

        nc.vector.memset(ones_bf[:, :], 1.0)
        # wq on the gpsimd SWDGE ring first (first matmuls need it with xt);
        # the rest of the weights queue behind the gpsimd-shard of xt loads.
        for c in range(EC):
            nc.gpsimd.dma_start(out=wq[c][:, :], in_=wq_ext[c * P:(c + 1) * P, :])

        # ---- assemble delayed^T: per delay-group shifted loads ----
        # Compute engines need 32-aligned partition bases, so zero the full
        # [0:dmax) margin per chunk first; the shifted DMAs (arbitrary
        # partition ranges are fine for DMA) overwrite [d:) per group and
        # program order resolves the overlap in the DMA's favor.  Loads
        # alternate over both HWDGE rings; ACT carries no compute during the
        # load window (Q/K/V psum copies all run on DVE) so both rings drain
        # back-to-back.
        dmax = [0] * EC
        chunk_groups: list[list[tuple]] = [[] for _ in range(EC)]
        for (r0, r1, dv) in groups:
            while r0 < r1:
                c = r0 // P
                seg_end = min(r1, (c + 1) * P)
                d = int(min(dv, S))
                dmax[c] = max(dmax[c], d)
                chunk_groups[c].append((r0 - c * P, seg_end - c * P, d))
                r0 = seg_end
        for c in range(EC):
            if dmax[c] > 0:
                nc.vector.memset(xt[c][:, 0:dmax[c]], 0.0)
        dma_rr = 0
        engs = (nc.sync, nc.scalar, nc.gpsimd)
        for c in range(EC):
            for (p0, p1, d) in chunk_groups[c]:
                if d < S:
                    eng = engs[dma_rr % 3]
                    dma_rr += 1
                    eng.dma_start(out=xt[c][p0:p1, d:S],
                                  in_=xT_ext[c * P + p0:c * P + p1, 0:S - d])
        for c in range(EC):
            nc.gpsimd.dma_start(out=wk[c][:, :], in_=wk_ext[c * P:(c + 1) * P, :])
        for c in range(EC):
            nc.gpsimd.dma_start(out=wv[c][:, :], in_=wv_ext[c * P:(c + 1) * P, :])
            nc.gpsimd.dma_start(out=wo[c][:, :], in_=wo_ext[c * P:(c + 1) * P, :])
            nc.gpsimd.dma_start(out=bo_sb[:, c:c + 1], in_=bo_ext[c * P:(c + 1) * P, :])

        # ---- Q^T, K^T: out[i,t] chunks; weights stationary ----
        for (w, dst, scale) in ((wq, qt_sb, INV_SQRT_HD), (wk, kt_sb, 1.0)):
            for ic in range(EC):
                for tt in range(NTT):
                    ps = psp.tile([P, TT], F32, name="mmps", tag="mmps", bufs=2)
                    for jc in range(EC):
                        nc.tensor.matmul(
                            ps[:, :],
                            lhsT=w[jc][:, ic * P:(ic + 1) * P],
                            rhs=xt[jc][:, tt * TT:(tt + 1) * TT],
                            start=(jc == 0),
                            stop=(jc == EC - 1),
                        )
                    if scale != 1.0:
                        nc.vector.tensor_scalar_mul(
                            dst[ic][:, tt * TT:(tt + 1) * TT], ps[:, :], scale)
                    else:
                        nc.vector.tensor_copy(
                            out=dst[ic][:, tt * TT:(tt + 1) * TT], in_=ps[:, :],
                        )

        # ---- V (normal orientation): xT stationary, wv moving ----
        for t in range(KC):
            for h in range(2):
                psv = psp.tile([P, VH], F32, name="mmps", tag="mmps", bufs=2)
                for jc in range(EC):
                    nc.tensor.matmul(
                        psv[:, :],
                        lhsT=xt[jc][:, t * P:(t + 1) * P],
                        rhs=wv[jc][:, h * VH:(h + 1) * VH],
                        start=(jc == 0),
                        stop=(jc == EC - 1),
                    )
                nc.vector.tensor_copy(out=v_sb[t][:, h * VH:(h + 1) * VH], in_=psv[:, :])

        # ---- attention, one 512-wide q-tile at a time ----
        for q in range(NQT):
            q0 = q * QT
            exp_tiles = []
            psS = psp.tile([P, QT], F32, name="scl", tag="scl", bufs=2)
            for kc in range(KC):
                ps = psp.tile([P, QT], F32, name="mmps", tag="mmps", bufs=2)
                for ic in range(EC):
                    nc.tensor.matmul(
                        ps[:, :],
                        lhsT=kt_sb[ic][:, kc * P:(kc + 1) * P],
                        rhs=qt_sb[ic][:, q0:q0 + QT],
                        start=(ic == 0),
                        stop=(ic == EC - 1),
                    )
                e = expp.tile([P, QT], BF16, name="exp", tag="exp", bufs=20)
                nc.scalar.activation(
                    out=e[:, :], in_=ps[:, :], func=mybir.ActivationFunctionType.Exp,
                )
                exp_tiles.append(e)
                nc.tensor.matmul(
                    psS[:, :], lhsT=ones_bf[:, :], rhs=e[:, :],
                    start=(kc == 0), stop=(kc == KC - 1),
                )

            # psS holds the denominators broadcast over all partitions; one
            # reciprocal lands the normalizer in SBUF (DVE can read only one
            # PSUM operand in the muls below).
            bc_sb = smallp.tile([P, QT], F32, name="bc_sb", tag="bc_sb", bufs=2)
            nc.vector.reciprocal(bc_sb[:, :], psS[:, :])

            pt_tiles = []
            for jc in range(EC):
                pv = psp.tile([P, QT], F32, name="pav", tag="pav", bufs=2)
                for kc in range(KC):
                    nc.tensor.matmul(
                        pv[:, :],
                        lhsT=v_sb[kc][:, jc * P:(jc + 1) * P],
                        rhs=exp_tiles[kc][:, :],
                        start=(kc == 0),
                        stop=(kc == KC - 1),
                    )
                pt = ptp.tile([P, QT], BF16, name="pt", tag="pt", bufs=8)
                nc.vector.tensor_mul(pt[:, :], pv[:, :], bc_sb[:, :])
                pt_tiles.append(pt)

            for ic in range(EC):
                po = psp.tile([P, QT], F32, name="po", tag="po", bufs=2)
                for jc in range(EC):
                    nc.tensor.matmul(
                        po[:, :],
                        lhsT=wo[jc][:, ic * P:(ic + 1) * P],
                        rhs=pt_tiles[jc][:, :],
                        start=(jc == 0),
                        stop=(jc == EC - 1),
                    )
                ot = otp.tile([P, QT], F32, name="ot", tag="ot", bufs=3)
                if q == NQT - 1:
                    nc.scalar.activation(
                        out=ot[:, :], in_=po[:, :],
                        func=mybir.ActivationFunctionType.Identity,
                        bias=bo_sb[:, ic:ic + 1])
                else:
                    nc.vector.tensor_scalar_add(ot[:, :], po[:, :],
                                                bo_sb[:, ic:ic + 1])
                oeng = nc.sync if ic % 2 == 0 else nc.scalar
                oeng.dma_start(out=out_ext[ic * P:(ic + 1) * P, q0:q0 + QT],
                               in_=ot[:, :])

    nc.compile()
    return nc


def _delay_groups(d_sorted):
    groups = []
    r0 = 0
    for r in range(1, len(d_sorted) + 1):
        if r == len(d_sorted) or d_sorted[r] != d_sorted[r0]:
            groups.append((r0, r, int(d_sorted[r0])))
            r0 = r
    return tuple(groups)


def kernel(**inputs) -> np.ndarray:
    global LAST_RESULT
    spikes = np.asarray(inputs["spikes"], dtype=np.float32)
    dw = np.asarray(inputs["delay_weights"]).reshape(-1).astype(np.int64)
    Wq = np.asarray(inputs["Wq"], dtype=np.float32)
    Wk = np.asarray(inputs["Wk"], dtype=np.float32)
    Wv = np.asarray(inputs["Wv"], dtype=np.float32)
    Wo = np.asarray(inputs["Wo"], dtype=np.float32)
    bo = np.asarray(inputs["bo"], dtype=np.float32)

    order = np.argsort(dw, kind="stable")
    groups = _delay_groups(dw[order])

    key = groups
    if key not in _BUILD_CACHE:
        _BUILD_CACHE[key] = _build(groups)
    nc = _BUILD_CACHE[key]

    bf = ml_dtypes.bfloat16
    wqT = np.ascontiguousarray(Wq[:, order].T).astype(bf)
    wkT = np.ascontiguousarray(Wk[:, order].T).astype(bf)
    wvT = np.ascontiguousarray(Wv[:, order].T).astype(bf)
    woT = np.ascontiguousarray(Wo.T).astype(bf)
    bo2 = np.ascontiguousarray(bo.reshape(E, 1))

    in_maps = []
    for b in range(B):
        xT = np.ascontiguousarray(spikes[b].T[order]).astype(bf)
        in_maps.append({"xT": xT, "wqT": wqT, "wkT": wkT, "wvT": wvT,
                        "woT": woT, "bo": bo2})

    LAST_RESULT = run_bass_kernel_spmd(
        nc, in_maps, core_ids=list(range(B)), trace=TRACE,
    )
    out = np.stack([LAST_RESULT.results[b]["out"].T for b in range(B)])
    return np.ascontiguousarray(out.astype(np.float32))
